# revision 26
# baseline (speedup 1.0000x reference)
"""Trainium2 Bass kernel for nn_BasePointSAModule (PointNet++ SA module).

Per-sample pipeline (one NeuronCore per batch element, B=8 across 8 cores):
  1. Farthest point sampling (1024 serial steps, exact fp32 arithmetic to
     reproduce the reference argmax indices bit-exactly).
  2. Ball query (radius 0.4, first-32-by-index) via PE distance matmuls and
     value-encoded top-32 selection (vector.max8 + match_replace rounds).
  3. Grouping gather (indirect DMA) + 3-layer 1x1-conv MLP (PE) with folded
     BN + ReLU epilogues (ACT) + max-pool over neighbors (DVE).

kernel(**inputs) takes the full batch, shards over 8 cores, returns
(new_xyz, new_features, indices) like the reference.
"""

import json

import numpy as np

import concourse.bass as bass
import concourse.mybir as mybir
from concourse.bass_utils import run_bass_kernel_spmd
from concourse.tile import TileContext

F32 = mybir.dt.float32
I32 = mybir.dt.int32
ALU = mybir.AluOpType
ACTF = mybir.ActivationFunctionType

N = 16384          # points per sample
NP = 128           # partitions
NF = N // NP       # flat point i = p*NF + f in the (128,128) layout
RADIUS2 = 0.4 * 0.4
K = 32             # nsample
CH = 64            # feature channels
TCOLS = 3 + CH     # grouped row: xyz + features
QW = 4096          # ball-query quarter width


def build_bass(npoint=1024, nblocks=8, split=True):
    M = nblocks * 128
    assert npoint == M, "center count must equal FPS steps"
    nc = bass.Bass(trn_type="TRN2")

    pts = nc.declare_dram_parameter("points_xyz", [N, 3], F32, isOutput=False)
    feat = nc.declare_dram_parameter("features", [CH, N], F32, isOutput=False)
    w0 = nc.declare_dram_parameter("W0", [64, 67], F32, isOutput=False)
    b0 = nc.declare_dram_parameter("b0", [64], F32, isOutput=False)
    s0 = nc.declare_dram_parameter("s0", [64], F32, isOutput=False)
    t0 = nc.declare_dram_parameter("t0", [64], F32, isOutput=False)
    w1 = nc.declare_dram_parameter("W1", [128, 64], F32, isOutput=False)
    b1 = nc.declare_dram_parameter("b1", [128], F32, isOutput=False)
    s1 = nc.declare_dram_parameter("s1", [128], F32, isOutput=False)
    t1 = nc.declare_dram_parameter("t1", [128], F32, isOutput=False)
    w2 = nc.declare_dram_parameter("W2", [256, 128], F32, isOutput=False)
    b2 = nc.declare_dram_parameter("b2", [256], F32, isOutput=False)
    s2 = nc.declare_dram_parameter("s2", [256], F32, isOutput=False)
    t2 = nc.declare_dram_parameter("t2", [256], F32, isOutput=False)

    o_xyz = nc.declare_dram_parameter("new_xyz", [M, 3], F32, isOutput=True)
    o_feat = nc.declare_dram_parameter("new_features", [256, M], F32, isOutput=True)
    o_idx = nc.declare_dram_parameter("indices", [M], I32, isOutput=True)
    o_dbg = nc.declare_dram_parameter("dbg_nbr", [M, K], I32, isOutput=True)

    tdram = nc.dram_tensor("T_table", [N, TCOLS], F32)

    v = nc.vector
    a = nc.scalar
    pe = nc.tensor
    g = nc.gpsimd
    sy = nc.sync

    with TileContext(nc) as tc:
        cst = tc.alloc_tile_pool(name="cst", bufs=1)
        blkb = tc.alloc_tile_pool(name="blkb", bufs=2)
        psA = tc.alloc_tile_pool(name="psA", bufs=1, space="PSUM")
        psB = tc.alloc_tile_pool(name="psB", bufs=1, space="PSUM")
        psH = tc.alloc_tile_pool(name="psH", bufs=1, space="PSUM")
        big1 = tc.alloc_tile_pool(name="big1", bufs=1)

        # ---------------- phase 0: constants + input staging ----------------
        xt = cst.tile([NP, NF], F32, tag="xt")
        yt = cst.tile([NP, NF], F32, tag="yt")
        zt = cst.tile([NP, NF], F32, tag="zt")
        pr = pts.rearrange("(p f) c -> p f c", p=NP)
        sy.dma_start(out=xt, in_=pr[:, :, 0])
        sy.dma_start(out=yt, in_=pr[:, :, 1])
        sy.dma_start(out=zt, in_=pr[:, :, 2])

        negx = cst.tile([NP, NF], F32, tag="negx")
        negy = cst.tile([NP, NF], F32, tag="negy")
        negz = cst.tile([NP, NF], F32, tag="negz")
        v.tensor_scalar_mul(negx, xt, -1.0)
        v.tensor_scalar_mul(negy, yt, -1.0)
        v.tensor_scalar_mul(negz, zt, -1.0)

        # iota_desc[p,f] = N - (p*NF + f)
        iota_desc = cst.tile([NP, NF], F32, tag="iota_desc")
        g.iota(iota_desc.bitcast(I32), pattern=[[-1, NF]], base=N,
               channel_multiplier=-NF)
        a.copy(iota_desc, iota_desc.bitcast(I32))

        ones128 = cst.tile([NP, NP], F32, tag="ones128")
        v.memset(ones128, 1.0)

        ident = cst.tile([NP, NP], F32, tag="ident")
        g.iota(ident.bitcast(I32), pattern=[[-1, NP]], base=0, channel_multiplier=1)
        v.tensor_scalar(ident, ident.bitcast(I32), 0, None, op0=ALU.is_equal)

        identrep = cst.tile([NP, 4 * NP], F32, tag="identrep")
        for kk in range(4):
            sy.dma_start(out=identrep[:, kk * NP:(kk + 1) * NP], in_=ident)
        dists = cst.tile([NP, NF], F32, tag="dists")
        v.memset(dists, 1e10)

        sqx = cst.tile([NP, NF], F32, tag="sqx")
        sqy = cst.tile([NP, NF], F32, tag="sqy")
        sqz = cst.tile([NP, NF], F32, tag="sqz")
        ssum = cst.tile([NP, NF], F32, tag="ssum")
        eq = cst.tile([NP, NF], F32, tag="eq")
        junk = cst.tile([NP, NF], F32, tag="junk")
        rowmax = cst.tile([NP, 1], F32, tag="rowmax")
        gmcol = cst.tile([NP, 1], F32, tag="gmcol")
        v.memset(gmcol, 0.0)
        cvbuf = cst.tile([NP, 4], F32, tag="cvbuf")
        bias_sb = cst.tile([NP, 3], F32, tag="bias_sb")
        mi_buf = cst.tile([1, M], F32, tag="mi_buf")
        cneg_flat = cst.tile([1, 3 * M], F32, tag="cneg_flat")

        # P5 rows: [1, px, py, pz, |p|^2] in flat point order
        p5 = cst.tile([5, N], F32, tag="p5")
        fr = pts.rearrange("(p f) c -> c (p f)", p=NP)
        v.memset(p5[0:1, :], 1.0)
        sy.dma_start(out=p5[1:2, :], in_=fr[0:1, :])
        sy.dma_start(out=p5[2:3, :], in_=fr[1:2, :])
        sy.dma_start(out=p5[3:4, :], in_=fr[2:3, :])
        psqx = cst.tile([NP, NF], F32, tag="psqx")
        psqs = cst.tile([NP, NF], F32, tag="psqs")
        a.activation(psqx, xt, ACTF.Square)
        v.tensor_tensor(psqs, yt, yt, op=ALU.mult)
        v.tensor_tensor(psqx, psqx, psqs, op=ALU.add)
        v.tensor_tensor(psqs, zt, zt, op=ALU.mult)
        v.tensor_tensor(psqx, psqx, psqs, op=ALU.add)
        sy.dma_start(out=p5[4:5, :], in_=psqx)

        # T table: [xyz | feat^T] rows per point
        sy.dma_start(out=tdram[:, 0:3], in_=pts[:, :])
        for c in range(N // NP):
            fchunk = blkb.tile([CH, NP], F32, tag="stage")
            sy.dma_start(out=fchunk, in_=feat[:, c * NP:(c + 1) * NP])
            ftp = psA.tile([NP, CH], F32, tag="tp")
            pe.transpose(ftp, fchunk, ident[0:CH, 0:CH])
            fts = blkb.tile([NP, CH], F32, tag="fts")
            a.copy(fts, ftp)
            sy.dma_start(out=tdram[c * NP:(c + 1) * NP, 3:TCOLS], in_=fts)

        def load_wT(wdram, rows, cols, tag):
            wsb = blkb.tile([rows, cols], F32, tag="stage")
            sy.dma_start(out=wsb, in_=wdram[:, :])
            wps = psA.tile([cols, rows], F32, tag="tp")
            pe.transpose(wps, wsb, ident[0:rows, 0:rows])
            wt = cst.tile([cols, rows], F32, tag=tag)
            a.copy(wt, wps)
            return wt

        w0t = load_wT(w0, 64, 67, "w0t")
        w1t = load_wT(w1, 128, 64, "w1t")
        w2ta = load_wT(w2[0:128, :], 128, 128, "w2ta")
        w2tb = load_wT(w2[128:256, :], 128, 128, "w2tb")

        def epi(bd, sd, td, n, tag):
            bt = blkb.tile([n, 1], F32, tag="stage3")
            st = cst.tile([n, 1], F32, tag=tag + "_s")
            tt = blkb.tile([n, 1], F32, tag="stage4")
            sy.dma_start(out=bt, in_=bd.rearrange("(n o) -> n o", o=1))
            sy.dma_start(out=st, in_=sd.rearrange("(n o) -> n o", o=1))
            sy.dma_start(out=tt, in_=td.rearrange("(n o) -> n o", o=1))
            bb = cst.tile([n, 1], F32, tag=tag + "_bb")
            v.tensor_tensor(bb, bt, st, op=ALU.mult)
            v.tensor_tensor(bb, bb, tt, op=ALU.add)
            return st, bb

        s0ap, B0ap = epi(b0, s0, t0, 64, "e0")
        s1ap, B1ap = epi(b1, s1, t1, 128, "e1")
        s2aap, B2aap = epi(b2[0:128], s2[0:128], t2[0:128], 128, "e2a")
        s2bap, B2bap = epi(b2[128:256], s2[128:256], t2[128:256], 128, "e2b")

        # ---------------- phase 1: FPS ----------------
        def fps_tail(t):
            # eq is (generically) a one-hot of the newly selected point, so
            # sum-accumulation extracts its (negated) coords + desc index.
            v.scalar_tensor_tensor(junk, in0=eq, scalar=0.0, in1=negx,
                                   op0=ALU.add, op1=ALU.mult,
                                   accum_out=cvbuf[:, 0:1])
            v.scalar_tensor_tensor(junk, in0=eq, scalar=0.0, in1=negy,
                                   op0=ALU.add, op1=ALU.mult,
                                   accum_out=cvbuf[:, 1:2])
            v.scalar_tensor_tensor(junk, in0=eq, scalar=0.0, in1=negz,
                                   op0=ALU.add, op1=ALU.mult,
                                   accum_out=cvbuf[:, 2:3])
            v.scalar_tensor_tensor(junk, in0=eq, scalar=0.0, in1=iota_desc,
                                   op0=ALU.add, op1=ALU.mult,
                                   accum_out=cvbuf[:, 3:4])
            nb = psA.tile([NP, 4], F32, tag="fps")
            pe.matmul(nb, lhsT=ones128, rhs=cvbuf, start=True, stop=True)
            a.copy(bias_sb, nb[:, 0:3])
            a.copy(mi_buf[0:1, t:t + 1], nb[0:1, 3:4])
            a.copy(cneg_flat[0:1, 3 * t:3 * t + 3], nb[0:1, 0:3])

        v.memset(eq, 0.0)
        v.memset(eq[0:1, 0:1], 1.0)
        fps_tail(0)

        for t in range(1, npoint):
            a.activation(sqx, xt, ACTF.Square, bias=bias_sb[:, 0:1])
            a.activation(sqy, yt, ACTF.Square, bias=bias_sb[:, 1:2])
            a.activation(sqz, zt, ACTF.Square, bias=bias_sb[:, 2:3])
            v.tensor_tensor(ssum, sqx, sqy, op=ALU.add)
            v.tensor_tensor(ssum, ssum, sqz, op=ALU.add)
            v.tensor_tensor(dists, dists, ssum, op=ALU.min)
            v.tensor_reduce(rowmax, dists, axis=mybir.AxisListType.X, op=ALU.max)
            rmT = psA.tile([1, NP], F32, tag="fps")
            pe.transpose(rmT, rowmax, ident)
            v.tensor_reduce(gmcol[0:1, 0:1], rmT, axis=mybir.AxisListType.X,
                            op=ALU.max)
            gmb = psA.tile([NP, 1], F32, tag="gmb")
            pe.matmul(gmb, lhsT=ones128, rhs=gmcol, start=True, stop=True)
            v.tensor_scalar(eq, dists, gmb[:, 0:1], None, op0=ALU.is_equal)
            fps_tail(t)

        idxrow = cst.tile([1, M], I32, tag="idxrow")
        v.tensor_scalar(idxrow, mi_buf, -1.0, float(N), op0=ALU.mult, op1=ALU.add)
        sy.dma_start(out=o_idx.rearrange("(o m) -> o m", o=1), in_=idxrow)

        # ---------------- phase 2 prep: C5 = [|c|^2, -2cx, -2cy, -2cz, 1] ----
        cnegT = cst.tile([3, M], F32, tag="cnegT")
        for c in range(3):
            sy.dma_start(out=cnegT[c:c + 1, :],
                         in_=cneg_flat.rearrange("o (m c) -> c o m", c=3)[c])
        c5 = cst.tile([5, M], F32, tag="c5")
        m2cT = cst.tile([3, M], F32, tag="m2cT")
        v.tensor_scalar_mul(m2cT, cnegT, 2.0)
        sy.dma_start(out=c5[1:4, :], in_=m2cT)
        sq3 = cst.tile([3, M], F32, tag="sq3")
        v.tensor_tensor(sq3, cnegT, cnegT, op=ALU.mult)
        for h in range(0, M, 512):
            w = min(512, M - h)
            csq = psA.tile([1, 512], F32, tag="tp")
            pe.matmul(csq[:, 0:w], lhsT=ones128[0:3, 0:1],
                      rhs=sq3[:, h:h + w], start=True, stop=True)
            a.copy(c5[0:1, h:h + w], csq[:, 0:w])
        onesrow = cst.tile([1, M], F32, tag="onesrow")
        v.memset(onesrow, 1.0)
        sy.dma_start(out=c5[4:5, :], in_=onesrow)

        # new_xyz output = -cneg (cneg no longer needed after C5)
        v.tensor_scalar_mul(cneg_flat, cneg_flat, -1.0)
        sy.dma_start(out=o_xyz.rearrange("(o m) c -> o (m c)", o=1), in_=cneg_flat)

        # ---------------- phases 2+3 per center-block ----------------
        vq = big1.tile([NP, QW], F32, tag="vq")
        for b in range(nblocks):
            cand = blkb.tile([NP, 128], F32, tag="cand")
            for h in range(4):
                g.iota(vq.bitcast(I32), pattern=[[-1, QW]], base=N - h * QW,
                       channel_multiplier=0)
                a.copy(vq, vq.bitcast(I32))
                for q in range(QW // 512):
                    ch = h * (QW // 512) + q
                    dq = psB.tile([NP, 512], F32, tag="dq")
                    pe.matmul(dq, lhsT=c5[:, b * NP:(b + 1) * NP],
                              rhs=p5[:, ch * 512:(ch + 1) * 512],
                              start=True, stop=True)
                    sl = slice(q * 512, (q + 1) * 512)
                    v.scalar_tensor_tensor(vq[:, sl], in0=dq, scalar=RADIUS2,
                                           in1=vq[:, sl], op0=ALU.is_le,
                                           op1=ALU.mult)
                for r in range(4):
                    m8 = blkb.tile([NP, 8], F32, tag="m8")
                    v.max(out=m8, in_=vq)
                    v.tensor_copy(cand[:, h * 32 + r * 8:h * 32 + r * 8 + 8], m8)
                    if r < 3:
                        v.match_replace(out=vq, in_to_replace=m8, in_values=vq,
                                        imm_value=0.0)
            val32 = blkb.tile([NP, K], F32, tag="val32")
            for r in range(4):
                m8b = blkb.tile([NP, 8], F32, tag="m8b")
                v.max(out=m8b, in_=cand)
                v.tensor_copy(val32[:, r * 8:r * 8 + 8], m8b)
                if r < 3:
                    v.match_replace(out=cand, in_to_replace=m8b, in_values=cand,
                                    imm_value=0.0)
            # pad empty slots (value 0) with slot-0's value
            emp = blkb.tile([NP, K], F32, tag="emp")
            v.tensor_scalar(emp, val32, 0.0, None, op0=ALU.is_equal)
            v.scalar_tensor_tensor(emp, in0=emp, scalar=0.0,
                                   in1=val32[:, 0:1].to_broadcast([NP, K]),
                                   op0=ALU.add, op1=ALU.mult)
            v.tensor_tensor(val32, val32, emp, op=ALU.add)
            gidx32 = blkb.tile([NP, K], I32, tag="gidx32")
            v.tensor_scalar(gidx32, val32, -1.0, float(N), op0=ALU.mult,
                            op1=ALU.add)

            sy.dma_start(out=o_dbg[b * NP:(b + 1) * NP, :], in_=gidx32)
            gg = big1.tile([NP, K, TCOLS], F32, tag="gg")
            for s in range(K):
                g.indirect_dma_start(
                    out=gg[:, s, :], out_offset=None, in_=tdram[:, :],
                    in_offset=bass.IndirectOffsetOnAxis(ap=gidx32[:, s:s + 1],
                                                        axis=0))

            gt = big1.tile([TCOLS, K * NP], F32, tag="gt")
            for s in range(K):
                gtp = psA.tile([TCOLS, NP], F32, tag="tp")
                pe.transpose(gtp, gg[:, s, :], ident)
                a.copy(gt[:, s * NP:(s + 1) * NP], gtp)

            btp = psA.tile([NP, 64], F32, tag="tp")
            pe.matmul(btp, lhsT=cnegT[:, b * NP:(b + 1) * NP], rhs=w0t[0:3, :],
                      start=True, stop=True)
            betaT = blkb.tile([NP, 64], F32, tag="betaT")
            a.copy(betaT, btp)

            pmaxa = blkb.tile([NP, NP], F32, tag="pmaxa")
            pmaxb = blkb.tile([NP, NP], F32, tag="pmaxb")
            v.memset(pmaxa, -3e38)
            v.memset(pmaxb, -3e38)
            for chk in range(K * NP // 512):
                sl = slice(chk * 512, (chk + 1) * 512)
                h0p = psH.tile([64, 512], F32, tag="h0p")
                pe.matmul(h0p, lhsT=w0t, rhs=gt[:, sl], start=True, stop=False)
                pe.matmul(h0p, lhsT=betaT, rhs=identrep,
                          start=False, stop=True)
                h0s = blkb.tile([64, 512], F32, tag="h0s")
                a.activation(h0s, h0p, ACTF.Relu, bias=B0ap, scale=s0ap)
                h1p = psH.tile([128, 512], F32, tag="h1p")
                pe.matmul(h1p, lhsT=w1t, rhs=h0s, start=True, stop=True)
                h1s = blkb.tile([128, 512], F32, tag="h1s")
                a.activation(h1s, h1p, ACTF.Relu, bias=B1ap, scale=s1ap)
                h2pa = psH.tile([128, 512], F32, tag="h2pa")
                pe.matmul(h2pa, lhsT=w2ta, rhs=h1s, start=True, stop=True)
                h2sa = blkb.tile([128, 512], F32, tag="h2sa")
                a.activation(h2sa, h2pa, ACTF.Relu, bias=B2aap, scale=s2aap)
                h2pb = psH.tile([128, 512], F32, tag="h2pb")
                pe.matmul(h2pb, lhsT=w2tb, rhs=h1s, start=True, stop=True)
                h2sb = blkb.tile([128, 512], F32, tag="h2sb")
                a.activation(h2sb, h2pb, ACTF.Relu, bias=B2bap, scale=s2bap)
                pa = blkb.tile([NP, NP], F32, tag="pa")
                pb = blkb.tile([NP, NP], F32, tag="pb")
                v.tensor_reduce(pa, h2sa.rearrange("p (s c) -> p c s", s=4),
                                axis=mybir.AxisListType.X, op=ALU.max)
                v.tensor_reduce(pb, h2sb.rearrange("p (s c) -> p c s", s=4),
                                axis=mybir.AxisListType.X, op=ALU.max)
                v.tensor_tensor(pmaxa, pmaxa, pa, op=ALU.max)
                v.tensor_tensor(pmaxb, pmaxb, pb, op=ALU.max)

            sy.dma_start(out=o_feat[0:128, b * NP:(b + 1) * NP], in_=pmaxa)
            sy.dma_start(out=o_feat[128:256, b * NP:(b + 1) * NP], in_=pmaxb)

        for p in (big1, psH, psB, psA, blkb, cst):
            p.release()

    if split:
        split_excess_waits(nc)
    return nc


def split_excess_waits(nc, max_waits=1):
    """This container's walrus caps sem-waits per instruction; hoist excess
    waits onto injected NoOps preceding the instruction on its engine."""
    m = json.loads(mybir.module_to_json_bytes(nc.m))
    ctr = 0
    for f in m["functions"]:
        for blk in f["blocks"]:
            out = []
            for ins in blk["instructions"]:
                si = ins.get("sync_info")
                waits = (si or {}).get("on_wait") or []
                if len(waits) > max_waits:
                    extra, keep = waits[:-max_waits], waits[-max_waits:]
                    for i in range(0, len(extra), max_waits):
                        ctr += 1
                        out.append({"engine": ins["engine"], "ins": [], "outs": [],
                                    "name": f"waitsplit_{ctr}", "opcode": "NoOp",
                                    "debug": ins.get("debug", 0),
                                    "sync_info": {"on_update": [],
                                                  "on_wait": extra[i:i + max_waits]}})
                    si["on_wait"] = keep
                out.append(ins)
            blk["instructions"] = out
    nc.m = mybir.module_from_json_bytes(json.dumps(m).encode())
    return ctr


_CACHE = {}


def _get_nc(npoint, nblocks):
    key = (npoint, nblocks)
    if key not in _CACHE:
        _CACHE[key] = build_bass(npoint, nblocks)
    return _CACHE[key]


def kernel(points_xyz, features, W0, b0, s0, t0, W1, b1, s1, t1,
           W2, b2, s2, t2, npoint=1024, trace=False):
    B = points_xyz.shape[0]
    nblocks = npoint // 128
    nc = _get_nc(npoint, nblocks)
    shared = dict(W0=W0, b0=b0, s0=s0, t0=t0, W1=W1, b1=b1, s1=s1, t1=t1,
                  W2=W2, b2=b2, s2=s2, t2=t2)
    in_maps = []
    for bidx in range(B):
        m = {"points_xyz": np.ascontiguousarray(points_xyz[bidx]),
             "features": np.ascontiguousarray(features[bidx])}
        m.update({k: np.ascontiguousarray(vv) for k, vv in shared.items()})
        in_maps.append(m)
    res = run_bass_kernel_spmd(nc, in_maps, core_ids=list(range(B)), trace=trace)
    kernel.last_results = res.results
    new_xyz = np.stack([res.results[i]["new_xyz"] for i in range(B)])
    new_feat = np.stack([res.results[i]["new_features"] for i in range(B)])
    idx = np.stack([res.results[i]["indices"] for i in range(B)])
    if trace:
        kernel.last_exec_ns = res.exec_time_ns
    return new_xyz, new_feat, idx.astype(np.int32)
